# revision 9
# baseline (speedup 1.0000x reference)
"""AxialAttention3D Trainium2 kernel (v2: fp8 DoubleRow attn@V + split exp
+ host-side softmax normalization).

Reference computes, for each of 3 weight branches (d/h/w), a full global
multi-head attention over the flattened 16^3 = 4096 spatial positions of
x (1, 128, 16, 16, 16), with 8 heads x dim_head 16, then
    out = gamma * (out_d + out_h + out_w) + x.

Sharding: 3 branches x 8 heads = 24 independent (branch, head) attention
units.  Core c computes head c of all 3 branches (3 units/core).  Each core
returns, per unit, the UNNORMALIZED projected output gamma*Wout_u(P_u V_u)
(128, 4096) plus the softmax denominator row (Σ_m p) per query; the host
divides, sums the 24 partials, and adds beff + residual x.

v2 changes over the bf16 baseline (463us measured; ACT-exp-bound at 370us
busy, PE 352us busy):
  - attn@V runs as fp8 DoubleRow matmuls: per 256-key double-tile, one
    matmul with stationary v^T pairs [128, 2, 32] (e4m3; col 0..15 = v,
    col 16 = ones for the denominator, 17..31 zero pad) and moving
    p pairs [128, 2, 512] (e5m2).  ~4x cheaper than the bf16 M=17
    matmuls (207us -> ~55us of PE wall).
  - exp split between Scalar(ACT) (native Exp -> e5m2) and Vector(DVE)
    via a Schraudolph log-domain trick: int8 byte = round(s*4log2e + b)
    IS the e5m2 encoding of C*e^s, C = 2^((b-60)/4).  The same C is
    folded into the ACT path (bias = ln C) and cancels in softmax.
    b is set per unit from a host-side score bound so bytes stay < 124
    (no e5m2 Inf/NaN) and ACT products stay < 57344.  Negative int8
    saturation lands on 0x80 = -0.0 (harmless flush of tiny weights).
  - normalization moved to the host: no reciprocal / broadcast-DMA /
    normalize-mul on device.  Per chunk: one acc->SBUF copy, then 3
    K=32 row-tiled out-projection matmuls DMAed straight from PSUM.
  - scores stay bf16 (K=16 row-tiled at 4 offsets, ~145ns per 128x512
    tile) -- fp8 DoubleRow would be LDWEIGHTS-bound there.
"""

import numpy as np


def _f8e4np():
    import ml_dtypes

    return ml_dtypes.float8_e4m3fn


def _bf16np():
    import ml_dtypes

    return ml_dtypes.bfloat16


HEADS = 8
DH = 16
C = 128
NCORES = 8

# GRP=2 key-tiles per scores/exp group = one DoubleRow attn@V per group.
_FULL = dict(
    MT=32, CHUNK=512, NCH=8, ACT16=9, LAG=2, COPY_DELAY=2, PROJ_DELAY=5,
    PJ_ACT=(True, False, True), K_ON_ACT=True, CP_ACT=(False, False, False)
)
_CACHE = {}

_LOG2E4 = 5.770780163555851  # 4 * log2(e): e5m2 byte slope per score unit


def _patch_tile_drain():
    """walrus in this env rejects >1 sync wait on one instruction; split the
    Tile kernel-tail drain's aggregated waits into one drain per wait."""
    import concourse.mybir as mybir
    from concourse.tile import TileContext, ScopedClock

    if getattr(TileContext, "_drain_split_patched", False):
        return

    def _drain_and_barrier_split(self, tick_clock, wait_clock):
        probe = self.nc.sync.drain()
        wait_clock.add_sem_waits(
            probe.ins, ScopedClock({None: tick_clock.global_clock})
        )
        si = probe.ins.sync_info
        waits = list(si.on_wait) if si is not None else []
        if len(waits) > 1:
            si.on_wait = [waits[0]]
            for w in waits[1:]:
                d = self.nc.sync.drain()
                d.ins.sync_info = mybir.SyncInfo(on_wait=[w], on_update=[])
        self.nc.all_engine_barrier()
        assert self.sems is not None
        popped = self.nc._tile_sem_poison_stack.pop()
        assert popped is self._sem_poison
        self.nc.clear_and_free_semaphores(list(self.sems.allocated().values()))
        self.nc.all_engine_barrier()

    TileContext._drain_and_barrier = _drain_and_barrier_split
    TileContext._drain_split_patched = True


def _split_multi_waits(nc):
    """walrus in this env allows at most ONE sync wait per instruction.
    Hoist extra waits onto same-engine NoOps inserted just before."""
    import concourse.mybir as mybir

    for f in nc.m.functions:
        for bb in f.blocks:
            new = []
            changed = False
            for inst in bb.instructions:
                si = inst.sync_info
                if si is not None and si.on_wait and len(si.on_wait) > 1:
                    waits = list(si.on_wait)
                    for j, w in enumerate(waits[:-1]):
                        nop = mybir.InstNoOp(
                            name=f"{inst.name}-w{j}",
                            engine=inst.engine,
                            sync_info=mybir.SyncInfo(on_wait=[w], on_update=[]),
                            bass_nofuse=True,
                        )
                        new.append(nop)
                    si.on_wait = [waits[-1]]
                    changed = True
                new.append(inst)
            if changed:
                bb.instructions = new


def build_nc(cfg=_FULL, split_waits=True):
    import concourse.bass as bass
    import concourse.mybir as mybir
    from concourse import tile

    _patch_tile_drain()

    f32 = mybir.dt.float32
    f32r = mybir.dt.float32r
    bf16 = mybir.dt.bfloat16
    f8e4 = mybir.dt.float8e4
    f8e5 = mybir.dt.float8e5
    u8 = mybir.dt.uint8
    Exp = mybir.ActivationFunctionType.Exp
    Copy = mybir.ActivationFunctionType.Copy
    DR = mybir.MatmulPerfMode.DoubleRow
    mult = mybir.AluOpType.mult
    add = mybir.AluOpType.add

    MT, CHUNK, NCH = cfg["MT"], cfg["CHUNK"], cfg["NCH"]
    ACT16 = cfg["ACT16"]  # of every 16 exp groups per (chunk,unit), on ACT
    N = MT * 128
    assert N == CHUNK * NCH
    ND = MT // 2  # double-tiles

    nc = bass.Bass("TRN2", target_bir_lowering=False, debug=False)

    x_d = nc.declare_dram_parameter("x", [C, N], bf16, isOutput=False)
    lq_d = [
        nc.declare_dram_parameter(f"lq{u}", [C, 128], bf16, isOutput=False)
        for u in range(3)
    ]
    lk_d = [
        nc.declare_dram_parameter(f"lk{u}", [C, 128], bf16, isOutput=False)
        for u in range(3)
    ]
    bq_d = [
        nc.declare_dram_parameter(f"bq{u}", [C, 1], f32, isOutput=False)
        for u in range(3)
    ]
    wv_d = nc.declare_dram_parameter("wv3", [C, 52], bf16, isOutput=False)
    wo_d = [
        nc.declare_dram_parameter(f"wo{u}", [32, 128], f32r, isOutput=False)
        for u in range(3)
    ]
    # exp constants, replicated down partitions: cols 0..2 = b_u (DVE byte
    # offset), cols 3..5 = ln(C_u) (ACT bias); C_u = 2^((b_u-60)/4)
    ec_d = nc.declare_dram_parameter("expc", [C, 8], f32, isOutput=False)
    y_u_d = [
        nc.declare_dram_parameter(f"y{u}", [C, N], f32, isOutput=True)
        for u in range(3)
    ]
    den_d = nc.declare_dram_parameter("dens", [4, N], f32, isOutput=True)

    with tile.TileContext(nc) as tc:
        with (
            tc.tile_pool(name="persist", bufs=1) as pp,
            tc.tile_pool(name="pt", bufs=4) as ptp,
            tc.tile_pool(name="scl", bufs=2) as sclp,
            tc.tile_pool(name="osb", bufs=3) as osbp,
            tc.tile_pool(name="big", bufs=2, space="PSUM") as bigp,
            tc.tile_pool(name="accp", bufs=2, space="PSUM") as accp,
            tc.tile_pool(name="projp", bufs=2, space="PSUM") as projp,
        ):
            # ---- persistent SBUF tensors ----
            x_sb = pp.tile([C, N], bf16, name="x_sb", tag="x")
            for cidx in range(NCH):
                nc.sync.dma_start(
                    x_sb[:, cidx * CHUNK : (cidx + 1) * CHUNK],
                    x_d[:, cidx * CHUNK : (cidx + 1) * CHUNK],
                )
            lq = [pp.tile([C, 128], bf16, name=f"lq{u}_sb", tag=f"lq{u}") for u in range(3)]
            lk = [pp.tile([C, 128], bf16, name=f"lk{u}_sb", tag=f"lk{u}") for u in range(3)]
            bq = [pp.tile([C, 1], f32, name=f"bq{u}_sb", tag=f"bq{u}") for u in range(3)]
            for u in range(3):
                nc.sync.dma_start(lq[u][:], lq_d[u][:])
                nc.sync.dma_start(lk[u][:], lk_d[u][:])
                nc.sync.dma_start(bq[u][:], bq_d[u][:])
            wv = pp.tile([C, 52], bf16, name="wv_sb", tag="wv")
            wo = [
                pp.tile([32, 128], f32r, name=f"wo{u}_sb", tag=f"wo{u}")
                for u in range(3)
            ]
            ec = pp.tile([C, 8], f32, name="ec_sb", tag="ec")
            nc.sync.dma_start(wv[:], wv_d[:])
            for u in range(3):
                nc.sync.dma_start(wo[u][:], wo_d[u][:])
            nc.sync.dma_start(ec[:], ec_d[:])

            qrep = [pp.tile([C, N], bf16, name=f"q{u}_sb", tag=f"q{u}") for u in range(3)]
            krep = [pp.tile([C, N], bf16, name=f"k{u}_sb", tag=f"k{u}") for u in range(3)]
            # v^T pairs: [C keys, (u, dtile, half, 32)] fp8e4; col 16 = ones
            vT = pp.tile([C, 3 * ND * 64], f8e4, name="vT_sb", tag="vT")

            def vT5(ap):
                return ap.rearrange("p (u d two c) -> p u d two c", u=3, d=ND, two=2)

            # one-time init: zero the v^T pad columns, write the ones column
            nc.vector.memset(vT[:], 0.0)
            nc.vector.memset(vT5(vT[:])[:, :, :, :, 16], 1.0)

            # ---- phase 0 (emitted partly up-front, partly dripped into
            # the phase-1 pipeline so the exp stream starts early) ----
            def emit_vt(t):
                ps = bigp.tile([C, 52], f32, name="vps", tag="scores")
                nc.tensor.matmul(
                    ps[:],
                    lhsT=x_sb[:, t * 128 : (t + 1) * 128],
                    rhs=wv[:],
                    start=True,
                    stop=True,
                )
                dst = vT5(vT[:])[:, :, t // 2, t % 2, 0:16]
                src = ps[:, 0:51].rearrange("p (u c) -> p u c", u=3)[:, :, 0:16]
                nc.vector.tensor_copy(dst, src)

            def emit_qk(u, cidx):
                cs, ce = cidx * CHUNK, (cidx + 1) * CHUNK
                psq = projp.tile([C, CHUNK], f32, name="qkps", tag="proj")
                nc.tensor.matmul(
                    psq[:], lhsT=lq[u][:], rhs=x_sb[:, cs:ce], start=True, stop=True
                )
                nc.vector.tensor_scalar_add(qrep[u][:, cs:ce], psq[:], bq[u][:])
                psk = projp.tile([C, CHUNK], f32, name="qkps", tag="proj")
                nc.tensor.matmul(
                    psk[:], lhsT=lk[u][:], rhs=x_sb[:, cs:ce], start=True, stop=True
                )
                if cfg.get("K_ON_ACT", True):
                    nc.scalar.activation(krep[u][:, cs:ce], psk[:], Copy)
                else:
                    nc.vector.tensor_copy(krep[u][:, cs:ce], psk[:])

            # Pre-block + drip, sized for 16 items per (chunk, unit).
            for t in range(4):
                emit_vt(t)
            emit_qk(0, 0)
            drip = (
                [("qk", (0, cidx)) for cidx in range(1, NCH)]
                + [("qk", (1, 0))]
                + [("vt", t) for t in range(4, MT)]
                + [("qk", (1, cidx)) for cidx in range(1, NCH)]
                + [("qk", (2, cidx)) for cidx in range(NCH)]
            )

            LAG = cfg["LAG"]
            COPY_DELAY = cfg["COPY_DELAY"]
            PROJ_DELAY = cfg["PROJ_DELAY"]

            items = []
            for cidx in range(NCH):
                for u in range(3):
                    for d in range(ND):
                        items.append((cidx, u, d))
            n_items = len(items)
            per_chunk = 3 * ND

            # exp-engine assignment: ACT16 of every 16 groups on ACT
            act_set = set()
            acc_f = 0.0
            for d in range(ND):
                acc_f += ACT16 / 16.0
                if acc_f >= 1.0 - 1e-9:
                    acc_f -= 1.0
                    act_set.add(d)

            acc_of = {}
            scl_of = {}
            pt_of_item = {}

            def emit_scores(idx):
                cidx, u, d = items[idx]
                cs, ce = cidx * CHUNK, (cidx + 1) * CHUNK
                sc = bigp.tile([C, CHUNK * 2], f32, name="sc_ps", tag="scores")
                for i in range(2):
                    t = 2 * d + i
                    r = t % 4
                    nc.tensor.matmul(
                        sc[:, i * CHUNK : (i + 1) * CHUNK],
                        lhsT=krep[u][32 * r : 32 * r + 16, t * 128 : (t + 1) * 128],
                        rhs=qrep[u][32 * r : 32 * r + 16, cs:ce],
                        start=True,
                        stop=True,
                        tile_position=(32 * r, 0),
                    )
                pt = ptp.tile([C, CHUNK * 2], f8e5, name="pt_sb", tag="pt")
                if d in act_set:
                    # p = exp(s + lnC) -> e5m2
                    nc.scalar.activation(
                        pt[:], sc[:], Exp, bias=ec[:, 3 + u : 4 + u]
                    )
                else:
                    # e5m2 byte = round(s * 4log2e + b_u); uint8 saturation
                    # at 0 = +0.0 flushes tiny weights
                    nc.vector.tensor_scalar(
                        pt[:].bitcast(u8),
                        sc[:],
                        _LOG2E4,
                        ec[:, u : u + 1],
                        mult,
                        add,
                    )
                pt_of_item[idx] = pt

            def emit_attnv(idx):
                cidx, u, d = items[idx]
                if (cidx, u) not in acc_of:
                    acc_of[(cidx, u)] = accp.tile(
                        [32, CHUNK], f32, name="acc_ps", tag="acc"
                    )
                acc = acc_of[(cidx, u)]
                pt = pt_of_item.pop(idx)
                nc.tensor.matmul(
                    acc[:],
                    lhsT=vT5(vT[:])[:, u, d],
                    rhs=pt[:].rearrange("p (two n) -> p two n", two=2),
                    start=(d == 0),
                    stop=(d == ND - 1),
                    perf_mode=DR,
                )

            def emit_copy(key):
                # stage one unit's acc ([16 dims + den + 15 zero] x CHUNK)
                # into SBUF for the projection matmul + den DMA
                cidx, u = key
                acc = acc_of.pop(key)
                scl = sclp.tile([32, CHUNK], f32r, name="scl_sb", tag="scl")
                if cfg.get("CP_ACT", (False, False, False))[u]:
                    nc.scalar.activation(scl[:], acc[:], Copy)
                else:
                    nc.vector.tensor_copy(scl[:], acc[:])
                cs, ce = cidx * CHUNK, (cidx + 1) * CHUNK
                nc.sync.dma_start(
                    den_d[u : u + 1, cs:ce], scl[16:17, :].bitcast(f32)
                )
                scl_of[key] = scl

            def emit_proj(key):
                cidx, u = key
                scl = scl_of.pop(key)
                cs, ce = cidx * CHUNK, (cidx + 1) * CHUNK
                pj = projp.tile([C, CHUNK], f32, name="pj_ps", tag="proj")
                nc.tensor.matmul(
                    pj[:], lhsT=wo[u][:], rhs=scl[:], start=True, stop=True
                )
                osb = osbp.tile([C, CHUNK], f32, name="osb_sb", tag="osb")
                if cfg.get("PJ_ACT", (True, False, True))[u]:
                    nc.scalar.activation(osb[:], pj[:], Copy)
                else:
                    nc.vector.tensor_copy(osb[:], pj[:])
                nc.sync.dma_start(y_u_d[u][:, cs:ce], osb[:])

            pending_copy = []
            pending_proj = []

            for idx in range(n_items + LAG + PROJ_DELAY + 1):
                while pending_copy and pending_copy[0][0] <= idx:
                    emit_copy(pending_copy.pop(0)[1])
                while pending_proj and pending_proj[0][0] <= idx:
                    emit_proj(pending_proj.pop(0)[1])
                for _ in range(3):
                    if drip:
                        kind, arg = drip.pop(0)
                        if kind == "vt":
                            emit_vt(arg)
                        else:
                            emit_qk(*arg)
                if idx < n_items:
                    emit_scores(idx)
                av = idx - LAG
                if 0 <= av < n_items:
                    emit_attnv(av)
                    ci, ui, di = items[av]
                    if di == ND - 1:
                        pending_copy.append((idx + COPY_DELAY, (ci, ui)))
                        pending_proj.append((idx + PROJ_DELAY, (ci, ui)))
            while pending_copy:
                emit_copy(pending_copy.pop(0)[1])
            while pending_proj:
                emit_proj(pending_proj.pop(0)[1])

    if split_waits:
        _split_multi_waits(nc)
    return nc


def host_prep(inputs, cfg=_FULL):
    """Slice/pack the full problem inputs into per-core input maps."""
    MT, CHUNK, NCH = cfg["MT"], cfg["CHUNK"], cfg["NCH"]
    N = MT * 128

    x = np.asarray(inputs["x"], dtype=np.float32)
    B = x.shape[0]
    assert B == 1
    xf = np.ascontiguousarray(x.reshape(C, -1))[:, :N]

    gamma0 = float(np.asarray(inputs["gamma"]).reshape(-1)[0])
    branches = [
        (
            np.asarray(inputs[f"w_qkv_{nm}"], dtype=np.float32),
            np.asarray(inputs[f"b_qkv_{nm}"], dtype=np.float32),
            np.asarray(inputs[f"w_out_{nm}"], dtype=np.float32),
            np.asarray(inputs[f"b_out_{nm}"], dtype=np.float32),
        )
        for nm in ("d", "h", "w")
    ]

    in_maps = []
    for h in range(NCORES):
        m = {"x": xf.astype(_bf16np()), "wv3": None}
        wv3 = np.zeros((C, 52), dtype=np.float32)
        expc = np.zeros((C, 8), dtype=np.float32)
        for u, (wqkv, bqkv, wout, bout) in enumerate(branches):
            wq = wqkv[h * DH : (h + 1) * DH, :]  # (16, 128)
            wk = wqkv[C + h * DH : C + (h + 1) * DH, :]
            wvu = wqkv[2 * C + h * DH : 2 * C + (h + 1) * DH, :]
            bqu = bqkv[h * DH : (h + 1) * DH]
            bku = bqkv[C + h * DH : C + (h + 1) * DH]

            lqm = np.zeros((C, 128), dtype=np.float32)
            lkm = np.zeros((C, 128), dtype=np.float32)
            bqm = np.zeros((C, 1), dtype=np.float32)
            for r in range(4):
                lqm[:, 32 * r : 32 * r + 16] = 0.5 * wq.T
                lkm[:, 32 * r : 32 * r + 16] = 0.5 * wk.T
                bqm[32 * r : 32 * r + 16, 0] = 0.5 * bqu
            m[f"lq{u}"] = lqm.astype(_bf16np())
            m[f"lk{u}"] = lkm.astype(_bf16np())
            m[f"bq{u}"] = bqm

            wv3[:, u * 17 : u * 17 + 16] = wvu.T  # col 16 stays 0
            wou = np.zeros((32, 128), dtype=np.float32)
            wou[0:16, :] = gamma0 * wout[:, h * DH : (h + 1) * DH].T
            m[f"wo{u}"] = wou

            # exp constants: bound max |score| to keep e5m2 out of Inf/NaN.
            # score = q'.k' with q' = 0.5(Wq x + bq), k' = 0.5(Wk x + bk).
            qp = 0.5 * (wq @ xf + bqu[:, None])  # (16, N)
            kp = 0.5 * (wk @ xf)  # bk dropped on device (softmax-invariant)
            s_bound = float(
                np.linalg.norm(qp, axis=0).max() * np.linalg.norm(kp, axis=0).max()
            )
            b_u = min(122.0 - _LOG2E4 * s_bound * 1.05, 60.0)
            # -0.215: centers the measured +3.9% log-domain rounding bias of
            # the DVE path so it matches the ACT path's C_u
            expc[:, u] = b_u - 0.2148
            expc[:, 3 + u] = (b_u - 60.0) * 0.25 * np.log(2.0)
        m["wv3"] = wv3.astype(_bf16np())
        m["expc"] = expc
        in_maps.append(m)
    return in_maps


def gather(results, inputs, cfg=_FULL):
    x = np.asarray(inputs["x"], dtype=np.float32)
    N = cfg["MT"] * 128

    gamma0 = float(np.asarray(inputs["gamma"]).reshape(-1)[0])
    beff = np.zeros(C, dtype=np.float64)
    for nm in ("d", "h", "w"):
        wqkv = np.asarray(inputs[f"w_qkv_{nm}"], dtype=np.float32)
        bqkv = np.asarray(inputs[f"b_qkv_{nm}"], dtype=np.float32)
        wout = np.asarray(inputs[f"w_out_{nm}"], dtype=np.float32)
        bout = np.asarray(inputs[f"b_out_{nm}"], dtype=np.float32)
        bv = bqkv[2 * C : 3 * C]
        beff += gamma0 * (wout.astype(np.float64) @ bv + bout)

    acc = np.zeros((C, N), dtype=np.float32)
    for r in results:
        dens = np.asarray(r["dens"], dtype=np.float32)
        for u in range(3):
            acc += np.asarray(r[f"y{u}"], dtype=np.float32) / dens[u : u + 1, :]
    out = acc + beff.astype(np.float32)[:, None] + x.reshape(C, -1)[:, :N]
    return out.reshape(x.shape).astype(np.float32)


def kernel(**inputs) -> np.ndarray:
    from concourse.bass_utils import run_bass_kernel_spmd

    if "nc" not in _CACHE:
        _CACHE["nc"] = build_nc(_FULL)
    nc = _CACHE["nc"]
    in_maps = host_prep(inputs, _FULL)
    res = run_bass_kernel_spmd(nc, in_maps, list(range(NCORES)))
    return gather(res.results, inputs, _FULL)
